# revision 13
# baseline (speedup 1.0000x reference)
"""NeuralCollapseLoss Trainium2 kernel.

loss = mean_b( max(EPS - ||features[b] - means[labels[b]]||_2, 0) )

Strategy (data-parallel over B across 8 NeuronCores):
  - Each core processes B/8 = 32768 rows of `features` (f32 [B, D]).
  - The [C, D] means table stays in DRAM on every core; per-sample mean rows
    are fetched with indirect (gather) DMA using the labels as row indices.
  - Per 512-row block: one 1 MiB feature DMA into a [128, 4*D] SBUF tile
    (partition p holds rows 4p..4p+3 of the block), 4 indirect gathers of
    [128, D] mean rows, one DVE subtract, and 4 ACT Square ops whose
    accum_out yields the per-sample sum of squared differences.
  - Epilogue: dist = sqrt(sumsq); hinge = relu(EPS - dist) with accum_out
    giving per-partition sums; GPSIMD partition-reduce -> scalar partial sum.
  - Host sums the 8 per-core partials and divides by B.
"""

from contextlib import ExitStack

import numpy as np

import concourse.bass as bass
import concourse.bacc as bacc
import concourse.tile as tile
from concourse import mybir
from concourse.bass_utils import run_bass_kernel_spmd

N_CORES = 8
B, D, C = 262144, 512, 1000
EPS = 5.0
P = 128
J = 4  # 128-row subtiles per block
ROWS_PER_BLOCK = P * J  # 512
ROWS_PER_CORE = B // N_CORES  # 32768

F32 = mybir.dt.float32
I32 = mybir.dt.int32


def build_nc(rows: int = ROWS_PER_CORE) -> bass.Bass:
    """Build the single-core program; SPMD-replicated across cores."""
    assert rows % ROWS_PER_BLOCK == 0
    n_blocks = rows // ROWS_PER_BLOCK
    cols = n_blocks * J  # accumulator columns; each partition owns `cols` samples

    nc = bacc.Bacc("TRN2")
    feats = nc.dram_tensor("features", [rows, D], F32, kind="ExternalInput")
    means = nc.dram_tensor("means", [C, D], F32, kind="ExternalInput")
    labels = nc.dram_tensor("labels", [P, cols], I32, kind="ExternalInput")
    out = nc.dram_tensor("out", [P, 1], F32, kind="ExternalOutput")

    with tile.TileContext(nc) as tc, ExitStack() as ctx:
        _kernel_body(ctx, tc, feats.ap(), means.ap(), labels.ap(), out.ap(), n_blocks)
    nc.compile()
    return nc


def _kernel_body(ctx, tc, feats, means, labels, out, n_blocks):
    nc = tc.nc
    cols = n_blocks * J

    # feature block n, partition p, free (j d) <- row n*512 + 4p + j
    f_blocks = feats.rearrange("(n p j) d -> n p (j d)", p=P, j=J)

    singles = ctx.enter_context(tc.tile_pool(name="singles", bufs=1))
    fpool = ctx.enter_context(tc.tile_pool(name="fpool", bufs=4))
    mpool = ctx.enter_context(tc.tile_pool(name="mpool", bufs=4))
    sqpool = ctx.enter_context(tc.tile_pool(name="sqpool", bufs=2, space="PSUM"))

    lab_sb = singles.tile([P, cols], I32)
    nc.sync.dma_start(out=lab_sb[:], in_=labels[:, :])

    # acc[p, n*J + j] = sum_d (f - m)^2 for sample n*512 + 4p + j
    acc = singles.tile([P, cols], F32)

    for n in range(n_blocks):
        f_tile = fpool.tile([P, J * D], F32)
        nc.sync.dma_start(out=f_tile[:], in_=f_blocks[n])

        # HW indirect DMA reads one index per partition, so gather each
        # 128-row subtile separately (same pattern as tile_scatter_add).
        m_tile = mpool.tile([P, J * D], F32)
        for j in range(J):
            nc.gpsimd.indirect_dma_start(
                out=m_tile[:, bass.ts(j, D)],
                out_offset=None,
                in_=means[:, :],
                in_offset=bass.IndirectOffsetOnAxis(
                    ap=lab_sb[:, n * J + j : n * J + j + 1], axis=0
                ),
            )

        # in-place: f_tile <- f_tile - m_tile
        nc.vector.tensor_tensor(
            out=f_tile[:], in0=f_tile[:], in1=m_tile[:], op=mybir.AluOpType.subtract
        )
        for j in range(J):
            sq = sqpool.tile([P, D], F32)
            nc.scalar.activation(
                out=sq[:],
                in_=f_tile[:, bass.ts(j, D)],
                func=mybir.ActivationFunctionType.Square,
                accum_out=acc[:, n * J + j : n * J + j + 1],
            )

    # Epilogue: dist, hinge, row-sum, partition-sum, store.
    eps_tile = singles.tile([P, 1], F32)
    nc.vector.memset(eps_tile[:], EPS)
    dist = singles.tile([P, cols], F32)
    nc.scalar.sqrt(dist[:], acc[:])
    hinge = singles.tile([P, cols], F32)
    partial = singles.tile([P, 1], F32)
    nc.scalar.activation(
        out=hinge[:],
        in_=dist[:],
        func=mybir.ActivationFunctionType.Relu,
        bias=eps_tile[:],
        scale=-1.0,
        accum_out=partial[:],
    )
    nc.sync.dma_start(out=out[:], in_=partial[:])


def _arrange_labels(lab_shard: np.ndarray) -> np.ndarray:
    """[rows] -> [128, cols] with lab[p, n*J+j] = lab_shard[n*512 + 4p + j]."""
    rows = lab_shard.shape[0]
    n_blocks = rows // ROWS_PER_BLOCK
    return np.ascontiguousarray(
        lab_shard.reshape(n_blocks, P, J).transpose(1, 0, 2).reshape(P, n_blocks * J)
    )


def make_in_maps(features, means, labels) -> list[dict]:
    features = np.asarray(features, dtype=np.float32)
    means = np.asarray(means, dtype=np.float32)
    labels = np.asarray(labels).astype(np.int32)
    assert features.shape == (B, D) and means.shape == (C, D) and labels.shape == (B,)
    in_maps = []
    for i in range(N_CORES):
        sl = slice(i * ROWS_PER_CORE, (i + 1) * ROWS_PER_CORE)
        in_maps.append(
            {
                "features": features[sl],
                "means": means,
                "labels": _arrange_labels(labels[sl]),
            }
        )
    return in_maps


def reduce_outputs(results: list[dict]) -> np.ndarray:
    total = sum(float(np.sum(r["out"])) for r in results)
    return np.float32(total / B)


def kernel(features, means, labels) -> np.ndarray:
    nc = build_nc(ROWS_PER_CORE)
    in_maps = make_in_maps(features, means, labels)
    res = run_bass_kernel_spmd(nc, in_maps, list(range(N_CORES))).results
    return reduce_outputs(res)
